# revision 20
# baseline (speedup 1.0000x reference)
"""DiT block with MoE — Trainium2 Bass/Tile kernel, 8-core SPMD.

Sharding: tokens split across 8 cores (cores 0-3 batch 0, cores 4-7 batch 1,
512 tokens each).  Activations are kept dim-major ([DIM, tokens]) on chip so
every projection / attention step is a dense chain of 128x128x512 bf16
matmuls with no on-device transposes:
  - layernorm / rmsnorm partition reductions -> ones-vector matmuls
  - per-token broadcasts -> K=1 ones matmuls
  - RoPE pair rotation -> host-side even/odd permutation of W_q/W_k output
    columns (scores are invariant to a shared per-head permutation of q,k)
  - softmax runs max-free (scores are O(5) for rmsnorm'd q,k), denominators
    via ones-matmuls
  - MoE top-k combine -> host-computed per-token expert mask rows; each
    expert output is (psum + b_e) * mask_e in one fused DVE op, accumulated
Self/cross-attention K,V shards are exchanged with AllGathers inside each
4-core batch group (K/V first, ctx-K/V second so self-attention can start
while the context gather is still in flight).  All weights are host-packed
so each weight DMA is one contiguous >=3KB-line block.  The residual stream
stays resident in SBUF in fp32.  FFN hidden activations and W2 run in fp8e4.
"""

import contextlib
import os

import numpy as np
import ml_dtypes

import concourse.bass as bass
import concourse.tile as tile
import concourse.mybir as mybir
from concourse import bacc, bass_utils
from concourse.bass import ds, ts

B, S, CTX, DIM, NH, FFN, NE, TOPK = 2, 2048, 512, 1536, 12, 6144, 4, 2
HD = DIM // NH          # 128
EPS = 1e-6
N_CORES = 8
CPB = N_CORES // B      # cores per batch = 4
TPC = S // CPB          # tokens per core = 512
CTXC = CTX // CPB       # ctx tokens per core = 128
KC = DIM // 128         # dim chunks = 12
FC = FFN // 128         # ffn chunks = 48
BF16 = mybir.dt.bfloat16
FP8 = mybir.dt.float8e4
F32 = mybir.dt.float32
bfnp = ml_dtypes.bfloat16
f8np = ml_dtypes.float8_e4m3

W2SCALE = 512.0         # host multiplies w2 by this before fp8 cast

ACTF = mybir.ActivationFunctionType
ALU = mybir.AluOpType

SZ_K = DIM * TPC
SZ_V = TPC * DIM
SZ_CK = DIM * CTXC
SZ_CV = CTXC * DIM
G1 = SZ_K + SZ_V        # first gather payload (self K,V)
G2 = SZ_CK + SZ_CV      # second gather payload (ctx K,V)

_VEC_NAMES = [
    "s1p", "sh1", "g1", "s2p", "sh2", "g2",
    "qb", "kb", "ob", "nqw", "nkw",
    "cqb", "ckb", "cob", "cnqw", "cnkw",
    "n3w", "n3b", "b2", "vb", "cvb",
    "mb0", "mb1", "mb2", "mb3",
]
_VBASE = {n: i * KC for i, n in enumerate(_VEC_NAMES)}
_VBASE["fb1"] = len(_VEC_NAMES) * KC
NV = len(_VEC_NAMES) * KC + FC


def build_bass():
    nc = bacc.Bacc("TRN2", target_bir_lowering=False, debug=False,
                   num_devices=N_CORES)

    def din(name, shape, dt):
        return nc.dram_tensor(name, shape, dt, kind="ExternalInput").ap()

    io = dict(
        xT=din("xT", [DIM, TPC], F32),
        ctxT=din("ctxT", [DIM, CTXC], BF16),
        cosT=din("cosT", [HD, TPC], BF16),
        sinT=din("sinT", [HD, TPC], BF16),
        vecs_d=din("vecs", [128, NV], F32),
        # packed projection weights: [o, p, k*128+i] = w.T[k*128+p, o*128+i]
        wq=din("wq", [KC, 128, DIM], BF16),
        wk=din("wk", [KC, 128, DIM], BF16),
        wv=din("wv", [DIM, DIM], BF16),
        wo=din("wo", [KC, 128, DIM], BF16),
        cwq=din("cwq", [KC, 128, DIM], BF16),
        cwk=din("cwk", [KC, 128, DIM], BF16),
        cwv=din("cwv", [DIM, DIM], BF16),
        cwo=din("cwo", [KC, 128, DIM], BF16),
        w1=din("w1", [FC, 128, DIM], BF16),
        w2=din("w2", [KC, 128, FFN], BF16),
        moew=din("moew", [NE, KC, 128, DIM], BF16),
        wall_d=din("wall", [NE, TPC], BF16),
        yT=nc.dram_tensor("yT", [DIM, TPC], F32, kind="ExternalOutput").ap(),
    )

    with tile.TileContext(nc) as tc:
        _emit(nc, tc, io)
    nc.compile()
    return nc


def _emit(nc, tc, io):
    xT, ctxT, cosT, sinT = io["xT"], io["ctxT"], io["cosT"], io["sinT"]
    vecs_d = io["vecs_d"]
    wq, wk, wv, wo = io["wq"], io["wk"], io["wv"], io["wo"]
    cwq, cwk, cwv, cwo = io["cwq"], io["cwk"], io["cwv"], io["cwo"]
    w1, w2, moew, wall_d = io["w1"], io["w2"], io["moew"], io["wall_d"]
    yT = io["yT"]

    ctx = contextlib.ExitStack()
    const = ctx.enter_context(tc.tile_pool(name="const", bufs=1))
    resid = ctx.enter_context(tc.tile_pool(name="resid", bufs=1))  # xr
    hp = ctx.enter_context(tc.tile_pool(name="hp", bufs=2))        # h/h3/h2
    big = ctx.enter_context(tc.tile_pool(name="big", bufs=2))      # kT/qT/...
    seq = ctx.enter_context(tc.tile_pool(name="seq", bufs=1))      # raw slot
    slab = ctx.enter_context(tc.tile_pool(name="slab", bufs=1))    # wv/cwv/ff
    crp = ctx.enter_context(tc.tile_pool(name="crp", bufs=1))      # ck tiles
    lnp = ctx.enter_context(tc.tile_pool(name="lnp", bufs=2))
    wtp = ctx.enter_context(tc.tile_pool(name="wtp", bufs=2))      # [128,DIM] w
    w2p = ctx.enter_context(tc.tile_pool(name="w2p", bufs=2))      # [128,FFN] w2
    sm = ctx.enter_context(tc.tile_pool(name="sm", bufs=3))
    att = ctx.enter_context(tc.tile_pool(name="att", bufs=2))
    exq = ctx.enter_context(tc.tile_pool(name="exq", bufs=2))
    tmp = ctx.enter_context(tc.tile_pool(name="tmp", bufs=6))
    bft = ctx.enter_context(tc.tile_pool(name="bft", bufs=3))
    vcp = ctx.enter_context(tc.tile_pool(name="vcp", bufs=1))
    psA = ctx.enter_context(tc.tile_pool(name="psA", bufs=4, space="PSUM"))
    psN = ctx.enter_context(tc.tile_pool(name="psN", bufs=2, space="PSUM"))
    psB = ctx.enter_context(tc.tile_pool(name="psB", bufs=2, space="PSUM"))
    dram = ctx.enter_context(tc.tile_pool(name="dram", bufs=1, space="DRAM"))

    # ---------------- constants
    vecs = const.tile([128, NV], F32, name="vecs_s")
    nc.sync.dma_start(vecs[:], vecs_d[:])

    def vcol(name, c):
        i = _VBASE[name] + c
        return vecs[:, i:i + 1]

    cos_t = const.tile([HD, TPC], BF16, name="cos_s")
    nc.sync.dma_start(cos_t[:], cosT[:])
    sin_t = const.tile([HD, TPC], BF16, name="sin_s")
    nc.sync.dma_start(sin_t[:], sinT[:])
    wallb = []
    for e in range(NE):
        wb = const.tile([128, TPC], BF16, tag=f"wallb{e}", name=f"wallb{e}")
        bc = bass.AP(tensor=wall_d.tensor, offset=e * TPC, ap=[[0, 128], [1, TPC]])
        nc.sync.dma_start(wb[:], bc)
        wallb.append(wb)
    ones_c = const.tile([128, 1], BF16, name="ones_c")
    nc.vector.memset(ones_c[:], 1.0)
    ones_r = const.tile([1, 128], BF16, name="ones_r")
    nc.vector.memset(ones_r[:], 1.0)
    ones_rf = const.tile([1, 128], F32, name="ones_rf")
    nc.vector.memset(ones_rf[:], 1.0)
    eps_t = const.tile([1, 1], F32, name="eps_t")
    nc.vector.memset(eps_t[:], EPS)

    SCL = float(1.0 / np.sqrt(HD))

    # resident residual stream (fp32, updated in place)
    xr = resid.tile([128, KC, TPC], F32, name="xr")
    for c in range(KC):
        nc.sync.dma_start(xr[:, c, :], xT[ts(c, 128), :])

    # ---------------- helpers
    def bcast_row(row_ap, n_tok):
        ps = psB.tile([128, n_tok], F32, tag="bc", name="bc_ps")
        nc.tensor.matmul(ps[:], ones_rf[:], row_ap, start=True, stop=True,
                         skip_group_check=True)
        return ps

    def layernorm(src, n_tok, out, sname=None, shname=None, wname=None,
                  bname=None):
        """mean/var over partitions via ones-matmuls; src(c) -> f32 AP."""
        ps_s = psN.tile([1, n_tok], F32, tag="nsum", name="ln_ps_s")
        ps_q = psN.tile([1, n_tok], F32, tag="nsum", name="ln_ps_q")
        xbs = []
        for c in range(KC):
            xb = lnp.tile([128, n_tok], BF16, tag="lnxb", name="lnxb")
            nc.vector.tensor_copy(xb[:], src(c))
            nc.tensor.matmul(ps_s[:], ones_c[:], xb[:], start=(c == 0),
                             stop=(c == KC - 1), skip_group_check=True)
            sq = lnp.tile([128, n_tok], BF16, tag="lnsq", name="lnsq")
            nc.vector.tensor_mul(sq[:], xb[:], xb[:])
            nc.tensor.matmul(ps_q[:], ones_c[:], sq[:], start=(c == 0),
                             stop=(c == KC - 1), skip_group_check=True)
        mean = sm.tile([1, n_tok], F32, tag="s", name="mean")
        nc.scalar.activation(mean[:], ps_s[:], ACTF.Copy, scale=1.0 / DIM)
        ex2 = sm.tile([1, n_tok], F32, tag="s", name="ex2")
        nc.scalar.activation(ex2[:], ps_q[:], ACTF.Copy, scale=1.0 / DIM)
        m2 = sm.tile([1, n_tok], F32, tag="s", name="m2")
        nc.vector.tensor_mul(m2[:], mean[:], mean[:])
        mb_ps = bcast_row(mean[:], n_tok)
        mb = tmp.tile([128, n_tok], F32, tag="f32t", name="mb")
        nc.vector.tensor_copy(mb[:], mb_ps[:])
        var = sm.tile([1, n_tok], F32, tag="s", name="var")
        nc.vector.tensor_sub(var[:], ex2[:], m2[:])
        std = sm.tile([1, n_tok], F32, tag="s", name="std")
        nc.scalar.activation(std[:], var[:], ACTF.Sqrt, bias=eps_t[:1, :])
        rstd = sm.tile([1, n_tok], F32, tag="s", name="rstd")
        nc.vector.reciprocal(rstd[:], std[:])
        rb_ps = bcast_row(rstd[:], n_tok)
        rb = tmp.tile([128, n_tok], F32, tag="f32t", name="rb")
        nc.vector.tensor_copy(rb[:], rb_ps[:])
        for c in range(KC):
            u = tmp.tile([128, n_tok], F32, tag="f32t", name="lnu")
            nc.vector.tensor_sub(u[:], src(c), mb[:])
            u2 = tmp.tile([128, n_tok], F32, tag="f32t", name="lnu2")
            nc.vector.tensor_mul(u2[:], u[:], rb[:])
            if sname is not None:
                nc.scalar.activation(out[:, c, :], u2[:], ACTF.Identity,
                                     bias=vcol(shname, c), scale=vcol(sname, c))
            else:
                nc.scalar.activation(out[:, c, :], u2[:], ACTF.Identity,
                                     bias=vcol(bname, c), scale=vcol(wname, c))

    def proj_dim_major(h_tile, w_d, n_tok, consumer, n_o=KC):
        """dim-major out: for each o-chunk, one packed weight DMA + KC mm."""
        for o in range(n_o):
            wt = wtp.tile([128, DIM], BF16, tag="w", name="wt")
            nc.sync.dma_start(wt[:], w_d[o, :, :])
            ps = psA.tile([128, n_tok], F32, tag="mm", name="proj_ps")
            for k in range(KC):
                nc.tensor.matmul(ps[:], wt[:, ts(k, 128)], h_tile[:, k, :],
                                 start=(k == 0), stop=(k == KC - 1),
                                 skip_group_check=True)
            consumer(o, ps)

    def rms_apply(raw, wname, n_tok, out):
        ps_q = psN.tile([1, n_tok], F32, tag="nsum", name="rms_ps")
        for c in range(KC):
            s = lnp.tile([128, n_tok], BF16, tag="lnsq", name="rmsq")
            nc.vector.tensor_mul(s[:], raw[:, c, :], raw[:, c, :])
            nc.tensor.matmul(ps_q[:], ones_c[:], s[:], start=(c == 0),
                             stop=(c == KC - 1), skip_group_check=True)
        ms = sm.tile([1, n_tok], F32, tag="s", name="rms_ms")
        nc.scalar.activation(ms[:], ps_q[:], ACTF.Sqrt, bias=eps_t[:1, :],
                             scale=1.0 / DIM)
        rstd = sm.tile([1, n_tok], F32, tag="s", name="rms_r")
        nc.vector.reciprocal(rstd[:], ms[:])
        rb_ps = bcast_row(rstd[:], n_tok)
        rb = tmp.tile([128, n_tok], F32, tag="f32t", name="rms_rb")
        nc.vector.tensor_copy(rb[:], rb_ps[:])
        for c in range(KC):
            u = tmp.tile([128, n_tok], F32, tag="f32t", name="rms_u")
            nc.vector.tensor_mul(u[:], raw[:, c, :], rb[:])
            nc.scalar.activation(out[:, c, :], u[:], ACTF.Identity,
                                 scale=vcol(wname, c))

    def rope_inplace(big_tile, n_tok):
        for c in range(KC):
            q = big_tile[:, c, :]
            qs = bft.tile([128, n_tok], BF16, tag="bft", name="rpswap")
            nc.sync.dma_start(qs[0:64, :], q[64:128, :])
            nc.sync.dma_start(qs[64:128, :], q[0:64, :])
            t1 = bft.tile([128, n_tok], BF16, tag="bft", name="rp1")
            nc.vector.tensor_mul(t1[:], q, cos_t[:, :n_tok])
            t2 = bft.tile([128, n_tok], BF16, tag="bft", name="rp2")
            nc.vector.tensor_mul(t2[:], qs[:], sin_t[:, :n_tok])
            nc.vector.tensor_add(q, t1[:], t2[:])

    # ================= stage 1: ln1 + modulation -> h
    h = hp.tile([128, KC, TPC], BF16, tag="h", name="h_t")
    layernorm(lambda c: xr[:, c, :], TPC, h, sname="s1p", shname="sh1")

    # ================= stage 2: k, v -> bounce + AllGather #1
    k_raw = seq.tile([128, KC, TPC], BF16, tag="seq", name="k_raw")

    def k_cons(o, ps):
        nc.scalar.activation(k_raw[:, o, :], ps[:], ACTF.Identity,
                             bias=vcol("kb", o))

    proj_dim_major(h, wk, TPC, k_cons)
    kT = big.tile([128, KC, TPC], BF16, tag="kv", name="kT")
    rms_apply(k_raw, "nkw", TPC, kT)
    rope_inplace(kT, TPC)

    inb1 = dram.tile([G1], BF16, name="inb1")
    gout1 = dram.tile([CPB, G1], BF16, name="gout1")
    inb2 = dram.tile([G2], BF16, name="inb2")
    gout2 = dram.tile([CPB, G2], BF16, name="gout2")

    # v: token-major via full-wv SBUF residency (one slab slot)
    wv_sb = slab.tile([128, KC, DIM], BF16, tag="slab", name="wv_sb")
    for k in range(KC):
        nc.sync.dma_start(wv_sb[:, k, :], wv[ts(k, 128), :])

    def tokmajor_proj(h_tile, wsb, out_cb, n_tok):
        """out[tok128, dim] = h.T @ W; bias folded into attention output
        (softmax rows sum to 1, so V-bias adds exactly b per output)."""
        n_tw = n_tok // 128
        for ob in range(3):
            for tw in range(n_tw):
                ps = psA.tile([128, 512], F32, tag="mm", name="vps")
                for k in range(KC):
                    nc.tensor.matmul(ps[:], h_tile[:, k, ts(tw, 128)],
                                     wsb[:, k, ds(ob * 512, 512)],
                                     start=(k == 0), stop=(k == KC - 1),
                                     skip_group_check=True)
                out_cb(tw, ob, ps)

    def v_cons(tw, ob, ps):
        vt = vcp.tile([128, 512], BF16, tag="vc", name="vtmp")
        nc.vector.tensor_copy(vt[:], ps[:])
        nc.sync.dma_start(
            bass.AP(tensor=inb1.tensor,
                    offset=inb1.offset + SZ_K + tw * 128 * DIM + ob * 512,
                    ap=[[DIM, 128], [1, 512]]),
            vt[:])

    tokmajor_proj(h, wv_sb, v_cons, TPC)

    off = 0
    for c in range(KC):
        nc.sync.dma_start(
            inb1[ds(off, 128 * TPC)].rearrange("(p t) -> p t", p=128),
            kT[:, c, :])
        off += 128 * TPC
    groups = [list(range(CPB)), list(range(CPB, 2 * CPB))]
    nc.gpsimd.collective_compute(
        "AllGather", ALU.bypass, replica_groups=groups,
        ins=[inb1.opt()], outs=[gout1.opt()])

    # ================= stage 3: ctx k/v -> bounce + AllGather #2
    ctx_sb = crp.tile([128, KC, CTXC], BF16, tag="ctx", name="ctx_sb")
    for c in range(KC):
        nc.sync.dma_start(ctx_sb[:, c, :], ctxT[ts(c, 128), :])
    ck_raw = crp.tile([128, KC, CTXC], BF16, tag="craw", name="ck_raw")

    def ck_cons(o, ps):
        nc.scalar.activation(ck_raw[:, o, :], ps[:], ACTF.Identity,
                             bias=vcol("ckb", o))

    proj_dim_major(ctx_sb, cwk, CTXC, ck_cons)
    rms_apply(ck_raw, "cnkw", CTXC, ck_raw)

    cwv_sb = slab.tile([128, KC, DIM], BF16, tag="slab", name="cwv_sb")
    for k in range(KC):
        nc.sync.dma_start(cwv_sb[:, k, :], cwv[ts(k, 128), :])

    def cv_cons(tw, ob, ps):
        vt = vcp.tile([128, 512], BF16, tag="vc", name="cvtmp")
        nc.vector.tensor_copy(vt[:], ps[:])
        nc.sync.dma_start(
            bass.AP(tensor=inb2.tensor,
                    offset=inb2.offset + SZ_CK + ob * 512,
                    ap=[[DIM, 128], [1, 512]]),
            vt[:])

    tokmajor_proj(ctx_sb, cwv_sb, cv_cons, CTXC)

    off = 0
    for c in range(KC):
        nc.sync.dma_start(
            inb2[ds(off, 128 * CTXC)].rearrange("(p t) -> p t", p=128),
            ck_raw[:, c, :])
        off += 128 * CTXC
    nc.gpsimd.collective_compute(
        "AllGather", ALU.bypass, replica_groups=groups,
        ins=[inb2.opt()], outs=[gout2.opt()])

    # ================= stage 4: q (overlaps gathers)
    q_raw = seq.tile([128, KC, TPC], BF16, tag="seq", name="q_raw")

    def q_cons(o, ps):
        nc.scalar.activation(q_raw[:, o, :], ps[:], ACTF.Identity,
                             bias=vcol("qb", o))

    proj_dim_major(h, wq, TPC, q_cons)
    qT = big.tile([128, KC, TPC], BF16, tag="kv", name="qT")
    rms_apply(q_raw, "nqw", TPC, qT)
    rope_inplace(qT, TPC)

    # ================= stage 5: self-attention
    def attention(q_big, n_kch, k_src, v_src, out_big, n_kt, vbname,
                  stage_len):
        """k_src(hh,ci)->staged [128, stage_len*n_kt]; v_src(hh,ci)->[128,128]."""
        for hh in range(NH):
            aps = psA.tile([128, TPC], F32, tag="mm", name="aps")
            dps = psN.tile([1, TPC], F32, tag="nsum", name="dps")
            for ci in range(n_kch):
                if ci % stage_len == 0:
                    kst, base = k_src(hh, ci), ci
                sps = psA.tile([128, TPC], F32, tag="mm", name="sps")
                nc.tensor.matmul(sps[:], kst[:, ts(ci - base, n_kt)],
                                 q_big[:, hh, :],
                                 start=True, stop=True, skip_group_check=True)
                ex = exq.tile([128, TPC], BF16, tag="ex", name="ex")
                nc.scalar.activation(ex[:], sps[:], ACTF.Exp, scale=SCL)
                vt = v_src(hh, ci)
                nc.tensor.matmul(aps[:], vt, ex[:], start=(ci == 0),
                                 stop=(ci == n_kch - 1), skip_group_check=True)
                nc.tensor.matmul(dps[:], ones_c[:], ex[:], start=(ci == 0),
                                 stop=(ci == n_kch - 1), skip_group_check=True)
            rec = sm.tile([1, TPC], F32, tag="s", name="rec")
            nc.vector.reciprocal(rec[:], dps[:])
            rb_ps = bcast_row(rec[:], TPC)
            rb = tmp.tile([128, TPC], F32, tag="f32t", name="arb")
            nc.vector.tensor_copy(rb[:], rb_ps[:])
            nc.vector.tensor_mul(out_big[:, hh, :], aps[:], rb[:])
            nc.vector.tensor_scalar_add(out_big[:, hh, :], out_big[:, hh, :],
                                        vcol(vbname, hh))

    def sa_k_src(hh, ci):
        kst = att.tile([128, 2 * TPC], BF16, tag="kst", name="kst")
        for j in range(2):
            s = ci // 4 + j
            src = bass.AP(tensor=gout1.tensor,
                          offset=gout1.offset + s * G1 + hh * 128 * TPC,
                          ap=[[TPC, 128], [1, TPC]])
            nc.sync.dma_start(kst[:, ts(j, TPC)], src)
        return kst

    sa_vst = {}

    def sa_v_src(hh, ci):
        s, w = ci // 4, ci % 4
        if w == 0:
            vst = att.tile([128, 4 * 128], BF16, tag="vst", name="vst")
            src = bass.AP(
                tensor=gout1.tensor,
                offset=gout1.offset + s * G1 + SZ_K + hh * 128,
                ap=[[DIM, 128], [128 * DIM, 4], [1, 128]])
            nc.sync.dma_start(vst[:], src)
            sa_vst[0] = vst
        return sa_vst[0][:, ts(w, 128)]

    attnT = big.tile([128, KC, TPC], BF16, tag="kv", name="attnT")
    attention(qT, CPB * 4, sa_k_src, sa_v_src, attnT, 128, 'vb', 8)

    # ================= stage 6: o-proj + gated residual (in-place xr)
    def o_cons(o, ps):
        t1 = tmp.tile([128, TPC], F32, tag="f32t", name="ot1")
        nc.vector.tensor_scalar(t1[:], ps[:], vcol("ob", o), vcol("g1", o),
                                ALU.add, ALU.mult)
        nc.vector.tensor_add(xr[:, o, :], t1[:], xr[:, o, :])

    proj_dim_major(attnT, wo, TPC, o_cons)

    # ================= stage 7: norm3 -> h3, cq, cross attention
    h3 = hp.tile([128, KC, TPC], BF16, tag="h", name="h3_t")
    layernorm(lambda c: xr[:, c, :], TPC, h3, wname="n3w", bname="n3b")
    cq_raw = seq.tile([128, KC, TPC], BF16, tag="seq", name="cq_raw")

    def cq_cons(o, ps):
        nc.scalar.activation(cq_raw[:, o, :], ps[:], ACTF.Identity,
                             bias=vcol("cqb", o))

    proj_dim_major(h3, cwq, TPC, cq_cons)
    cqT = big.tile([128, KC, TPC], BF16, tag="kv", name="cqT")
    rms_apply(cq_raw, "cnqw", TPC, cqT)

    def ca_k_src(hh, ci):
        kst = att.tile([128, CPB * CTXC], BF16, tag="ckst", name="ckst")
        for s in range(CPB):
            src = bass.AP(tensor=gout2.tensor,
                          offset=gout2.offset + s * G2 + hh * 128 * CTXC,
                          ap=[[CTXC, 128], [1, CTXC]])
            nc.sync.dma_start(kst[:, ts(s, CTXC)], src)
        return kst

    ca_vst = {}

    def ca_v_src(hh, ci):
        if ci == 0:
            vst = att.tile([128, CPB * 128], BF16, tag="cvst", name="cvst")
            src = bass.AP(tensor=gout2.tensor,
                          offset=gout2.offset + SZ_CK + hh * 128,
                          ap=[[DIM, 128], [G2, CPB], [1, 128]])
            nc.sync.dma_start(vst[:], src)
            ca_vst[0] = vst
        return ca_vst[0][:, ts(ci, 128)]

    cattnT = big.tile([128, KC, TPC], BF16, tag="kv", name="cattnT")
    attention(cqT, CPB, ca_k_src, ca_v_src, cattnT, CTXC, 'cvb', CPB)

    # ================= stage 8: ca o-proj + residual (in-place xr)
    def co_cons(o, ps):
        t1 = tmp.tile([128, TPC], F32, tag="f32t", name="cot1")
        nc.vector.tensor_scalar_add(t1[:], ps[:], vcol("cob", o))
        nc.vector.tensor_add(xr[:, o, :], t1[:], xr[:, o, :])

    proj_dim_major(cattnT, cwo, TPC, co_cons)

    # ================= stage 9: ln2 -> h2
    h2 = hp.tile([128, KC, TPC], BF16, tag="h", name="h2_t")
    layernorm(lambda c: xr[:, c, :], TPC, h2, sname="s2p", shname="sh2")

    # ================= stage 10: ffn + moe, single pass over 512 tokens
    ff = slab.tile([128, FC, TPC], BF16, tag="slab", name="ff")
    for o in range(FC):
        wt = wtp.tile([128, DIM], BF16, tag="w", name="w1t")
        nc.sync.dma_start(wt[:], w1[o, :, :])
        ps = psA.tile([128, TPC], F32, tag="mm", name="ffps")
        for k in range(KC):
            nc.tensor.matmul(ps[:], wt[:, ts(k, 128)], h2[:, k, :],
                             start=(k == 0), stop=(k == KC - 1),
                             skip_group_check=True)
        nc.scalar.activation(ff[:, o, :], ps[:], ACTF.Gelu_apprx_tanh,
                             bias=vcol("fb1", o))

    for o in range(KC):
        ps = psA.tile([128, TPC], F32, tag="mm", name="w2ps")
        for half in range(2):
            w2t = w2p.tile([128, FFN // 2], BF16, tag="w2", name="w2t")
            nc.sync.dma_start(w2t[:], w2[o, :, ds(half * (FFN // 2), FFN // 2)])
            for kk in range(FC // 2):
                k = half * (FC // 2) + kk
                nc.tensor.matmul(ps[:], w2t[:, ts(kk, 128)], ff[:, k, :],
                                 start=(k == 0), stop=(k == FC - 1),
                                 skip_group_check=True)
        acc = tmp.tile([128, TPC], F32, tag="f32t", name="macc")
        nc.vector.tensor_scalar_add(acc[:], ps[:], vcol("b2", o))
        for e in range(NE):
            met = wtp.tile([128, DIM], BF16, tag="w", name="moet")
            nc.sync.dma_start(met[:], moew[e, o, :, :])
            pse = psA.tile([128, TPC], F32, tag="mm", name="pse")
            for k in range(KC):
                nc.tensor.matmul(pse[:], met[:, ts(k, 128)], h2[:, k, :],
                                 start=(k == 0), stop=(k == KC - 1),
                                 skip_group_check=True)
            te = tmp.tile([128, TPC], F32, tag="f32t", name="te")
            nc.vector.scalar_tensor_tensor(te[:], pse[:], vcol(f"mb{e}", o),
                                           wallb[e][:],
                                           ALU.add, ALU.mult)
            acc2 = tmp.tile([128, TPC], F32, tag="f32t", name="macc2")
            nc.vector.tensor_add(acc2[:], acc[:], te[:])
            acc = acc2
        t1 = tmp.tile([128, TPC], F32, tag="f32t", name="yt1")
        nc.vector.tensor_scalar_mul(t1[:], acc[:], vcol("g2", o))
        yc = tmp.tile([128, TPC], F32, tag="f32t", name="yc")
        nc.vector.tensor_add(yc[:], t1[:], xr[:, o, :])
        nc.sync.dma_start(yT[ts(o, 128), :], yc[:])

    ctx.close()


# -------------------------------------------------------------- host prep
def _rope_perm():
    p = np.arange(DIM).reshape(NH, HD)
    return np.concatenate([p[:, 0::2], p[:, 1::2]], axis=1).reshape(-1)


def _pack(wT, n_o):
    """[K_in*128, n_o*128] -> [n_o, 128, K_in*128] with
    R[o, p, k*128+i] = wT[k*128+p, o*128+i]; contiguous per-o DMA blocks."""
    k_in = wT.shape[0] // 128
    return np.ascontiguousarray(
        wT.reshape(k_in, 128, n_o, 128).transpose(2, 1, 0, 3)
        .reshape(n_o, 128, k_in * 128))


def prep_inputs(inputs):
    f = lambda a: np.asarray(a, dtype=np.float32)
    x = f(inputs["x"])
    context = f(inputs["context"])
    t_mod = f(inputs["t_mod"])
    freqs_cos = f(inputs["freqs_cos"])
    freqs_sin = f(inputs["freqs_sin"])
    ew = f(inputs["expert_weights"])
    idx = np.asarray(inputs["top_k_indices"])
    modulation = f(inputs["modulation"])

    perm = _rope_perm()

    def wT(a):
        return np.ascontiguousarray(f(a).T)

    wq_h = _pack(f(inputs["sa_q_w"])[perm].T, KC).astype(bfnp)
    wk_h = _pack(f(inputs["sa_k_w"])[perm].T, KC).astype(bfnp)
    wv_h = wT(inputs["sa_v_w"]).astype(bfnp)
    wo_h = _pack(wT(inputs["sa_o_w"]), KC).astype(bfnp)
    cwq_h = _pack(wT(inputs["ca_q_w"]), KC).astype(bfnp)
    cwk_h = _pack(wT(inputs["ca_k_w"]), KC).astype(bfnp)
    cwv_h = wT(inputs["ca_v_w"]).astype(bfnp)
    cwo_h = _pack(wT(inputs["ca_o_w"]), KC).astype(bfnp)
    w1_h = _pack(wT(inputs["ffn_w1"]), FC).astype(bfnp)
    w2_h = _pack(wT(inputs["ffn_w2"]), KC).astype(bfnp)
    moew_h = np.stack([
        _pack(np.ascontiguousarray(f(inputs["moe_w"])[e].T), KC)
        for e in range(NE)]).astype(bfnp)
    moeb = f(inputs["moe_b"])

    mod = modulation + t_mod
    cosA = np.concatenate([freqs_cos.T, freqs_cos.T], 0).astype(bfnp)
    sinA = np.concatenate([-freqs_sin.T, freqs_sin.T], 0).astype(bfnp)

    in_maps, metas = [], []
    for c in range(N_CORES):
        b, i = c // CPB, c % CPB
        tok = slice(i * TPC, (i + 1) * TPC)
        ctok = slice(i * CTXC, (i + 1) * CTXC)
        vecs = np.zeros((128, NV), np.float32)

        def setv(name, arr):
            n = len(arr) // 128
            vecs[:, _VBASE[name]:_VBASE[name] + n] = arr.reshape(n, 128).T

        m = mod[b]
        setv("s1p", 1.0 + m[1]); setv("sh1", m[0]); setv("g1", m[2])
        setv("s2p", 1.0 + m[4]); setv("sh2", m[3]); setv("g2", m[5])
        setv("qb", f(inputs["sa_q_b"])[perm])
        setv("kb", f(inputs["sa_k_b"])[perm])
        setv("ob", f(inputs["sa_o_b"]))
        setv("nqw", f(inputs["sa_nq_w"])[perm])
        setv("nkw", f(inputs["sa_nk_w"])[perm])
        setv("cqb", f(inputs["ca_q_b"])); setv("ckb", f(inputs["ca_k_b"]))
        setv("cob", f(inputs["ca_o_b"]))
        setv("cnqw", f(inputs["ca_nq_w"])); setv("cnkw", f(inputs["ca_nk_w"]))
        setv("n3w", f(inputs["norm3_w"])); setv("n3b", f(inputs["norm3_b"]))
        setv("b2", f(inputs["ffn_b2"])); setv("fb1", f(inputs["ffn_b1"]))
        for e in range(NE):
            setv(f"mb{e}", moeb[e])

        setv("vb", f(inputs["sa_v_b"])); setv("cvb", f(inputs["ca_v_b"]))

        wall = np.zeros((NE, TPC), np.float32)
        iw = idx[b, tok]
        eww = ew[b, tok]
        for kk in range(TOPK):
            np.add.at(wall, (iw[:, kk], np.arange(TPC)), eww[:, kk])

        in_maps.append({
            "xT": np.ascontiguousarray(x[b, tok].T),
            "ctxT": np.ascontiguousarray(context[b, ctok].T).astype(bfnp),
            "cosT": np.ascontiguousarray(cosA[:, tok]),
            "sinT": np.ascontiguousarray(sinA[:, tok]),
            "vecs": vecs,
            "wq": wq_h, "wk": wk_h, "wv": wv_h, "wo": wo_h,
            "cwq": cwq_h, "cwk": cwk_h, "cwv": cwv_h, "cwo": cwo_h,
            "w1": w1_h, "w2": w2_h, "moew": moew_h,
            "wall": wall.astype(bfnp),
        })
        metas.append((b, i))
    return in_maps, metas


_NC_CACHE = {}


def get_nc():
    if "nc" not in _NC_CACHE:
        _NC_CACHE["nc"] = build_bass()
    return _NC_CACHE["nc"]


def run(in_maps):
    nc = get_nc()
    return bass_utils.run_bass_kernel_spmd(
        nc, in_maps, core_ids=list(range(N_CORES)), trace=False)


def kernel(**inputs):
    in_maps, metas = prep_inputs(inputs)
    res = run(in_maps)
    out = np.zeros((B, S, DIM), np.float32)
    for c in range(N_CORES):
        b, i = metas[c]
        out[b, i * TPC:(i + 1) * TPC] = np.asarray(
            res.results[c]["yT"], dtype=np.float32).T
    kernel.last_results = res
    return out
